# revision 27
# baseline (speedup 1.0000x reference)
"""Trainium2 Bass kernel for nn_Attention_5480378270188.

Single-layer attention: q/k/v linear projections (torch Linear convention),
scores = q @ k^T (no 1/sqrt(d) scale), additive -1e9 mask, softmax over keys,
out = weights @ v.

Shapes (hardcoded): B=8, N=M=2048, D_MODEL=D_K=D_V=1024, fp32 inputs.

Sharding: data-parallel over batch — core b computes batch element b.
mask / W / biases are replicated to all 8 cores. No collectives.

On-device dtype strategy: all TensorE operands fp16 (full PE rate), fp32 PSUM
accumulation, softmax entirely in fp32. Operand transposes run on the DMA
X-bar (16-bit) out of an fp16 DRAM bounce produced by HWDGE staging + an
on-chip DVE/ACT cast — the PE does nothing but matmuls. bq/bk are applied
on-device (per-partition ACT bias fused into the projection PSUM->SBUF
copies). bv is applied on the host: softmax rows sum to 1, so
softmax(s) @ (v + bv) == softmax(s) @ v + bv exactly.
"""

import sys

for _p in ("/opt/trn_rl_repo", "/opt/pypackages"):
    if _p not in sys.path:
        sys.path.insert(0, _p)

from contextlib import ExitStack

import numpy as np

import concourse.bass as bass
import concourse.tile as tile
from concourse import bacc, mybir
from concourse.bass import ds, ts
from concourse.bass_utils import run_bass_kernel_spmd
from concourse.masks import make_identity

P = 128
B = 8
N = 2048  # queries
M = 2048  # keys
D = 1024  # d_model
DK = 1024  # key/query dim
DV = 1024  # value dim
F = 512  # matmul moving free dim
DT = mybir.dt.float16
F32 = mybir.dt.float32
I32 = mybir.dt.int32
I8 = mybir.dt.int8

NEG = -1.0e9

N_BLOCKS = N // P  # 16
M_BLOCKS = M // P  # 16
D_O = D // P  # 8
DK_O = DK // P  # 8
N_MEGA = N // F  # 4 query mega-blocks (512 rows)
M_GRP = M // F  # 4 key groups (512 rows)
SC_CHUNKS = M // F  # 4 score chunks per row-block
PV_CHUNKS = DV // F  # 2


def build():
    nc = bacc.Bacc("TRN2", target_bir_lowering=False, debug=False)

    querys_e = nc.dram_tensor("querys", [N, D], F32, kind="ExternalInput").ap()
    keys_e = nc.dram_tensor("keys", [M, D], F32, kind="ExternalInput").ap()
    values_e = nc.dram_tensor("values", [M, D], F32, kind="ExternalInput").ap()
    mask_e = nc.dram_tensor("mask", [N, M], I32, kind="ExternalInput").ap()
    Wq_e = nc.dram_tensor("Wq", [DK, D], F32, kind="ExternalInput").ap()
    Wk_e = nc.dram_tensor("Wk", [DK, D], F32, kind="ExternalInput").ap()
    Wv_e = nc.dram_tensor("Wv", [DV, D], F32, kind="ExternalInput").ap()
    bq_e = nc.dram_tensor("bq", [DK], F32, kind="ExternalInput").ap()
    bk_e = nc.dram_tensor("bk", [DK], F32, kind="ExternalInput").ap()
    out_e = nc.dram_tensor("out", [N, DV], F32, kind="ExternalOutput").ap()

    with tile.TileContext(nc) as tc, ExitStack() as ctx:
        const = ctx.enter_context(tc.tile_pool(name="const", bufs=1))
        persist = ctx.enter_context(tc.tile_pool(name="persist", bufs=1))
        dram = ctx.enter_context(tc.tile_pool(name="dram", bufs=1, space="DRAM"))
        psSC = ctx.enter_context(tc.tile_pool(name="psSC", bufs=3, space="PSUM"))
        psPV = ctx.enter_context(tc.tile_pool(name="psPV", bufs=1, space="PSUM"))

        ident16 = const.tile([P, P], DT, tag="id16")
        make_identity(nc, ident16[:])

        bq_sb = const.tile([P, DK_O], F32, tag="bq")
        nc.sync.dma_start(bq_sb[:], bq_e.rearrange("(o p) -> p o", p=P))
        bk_sb = const.tile([P, DK_O], F32, tag="bk")
        nc.sync.dma_start(bk_sb[:], bk_e.rearrange("(o p) -> p o", p=P))

        # persistent fp16 operands for the attention matmuls
        kT_sb = persist.tile([P, DK_O, M], DT, tag="kT")  # [dk_i, dk_o, m]
        v_sb = persist.tile([P, M_BLOCKS, DV], DT, tag="v")  # [m_i, m_o, dv]
        qT_sb = persist.tile([P, DK_O, N], DT, tag="qT")  # [dk_i, dk_o, n]

        # ---------------- Phase A: all projections ----------------
        with (
            tc.tile_pool(name="phW", bufs=1) as pw,
            tc.tile_pool(name="phA", bufs=5) as pa,
            tc.tile_pool(name="phT", bufs=2) as pact,
        ):
            # Transposed fp16 operands: HWDGE stage-in (fp32) -> on-chip
            # cast (DVE/ACT) -> SBUF->SBUF X-bar transpose per 128-row tile,
            # alternating between the two HWDGE queues. No DRAM bounce.
            def load_T(src_rows, dst, rows):
                """src_rows: [rows, D] fp32 DRAM -> dst [P, rows//P, D_O, P] fp16.

                Per 512 rows: one SWDGE cast-DMA (fp32->fp16 in flight) into a
                staging tile, then PE transposes (fp16, batched 4 per PSUM
                tile) with ACT/DVE copy-back. No X-bar: every DMA in the
                machine serializes globally against X-bar-transpose DMAs
                (~10us dead time per mode transition), which starves the PE;
                PE transposes cost ~130ns each and overlap the DMA stream.
                """
                for ch in range(rows // F):
                    src = src_rows[ds(ch * F, F), :]
                    st16 = pa.tile([P, 4, D], DT, tag="st16")
                    nc.gpsimd.dma_start(
                        st16[:], src.rearrange("(ro p) d -> p ro d", p=P)
                    )
                    for rt in range(4):
                        for h in range(2):
                            ps = psSC.tile([P, 4, P], DT, tag="ps_sc", name="ps_t")
                            for j in range(4):
                                db = h * 4 + j
                                nc.tensor.transpose(
                                    ps[:, j, :],
                                    st16[:, rt, ts(db, P)],
                                    ident16[:],
                                )
                            nc.any.tensor_copy(
                                dst[:, ch * 4 + rt, ds(h * 4, 4), :], ps[:]
                            )

            def make_wT(w_ext, tag):
                # [dk_i_rowtile(=dko), d_o, dk_i] — weight tile for (do, dko)
                # is wT[:, dko, do, :]
                wT = pw.tile([P, DK_O, D_O, P], DT, tag=f"{tag}T")
                load_T(w_ext[:], wT, DK)
                return wT

            WqT_sb = make_wT(Wq_e, "wq")

            WkT_sb = None
            WvT_sb = None

            # q projections (fill the PE while k/v staging is in flight)
            for g in range(N_MEGA):
                qTt = pact.tile([P, 4, D_O, P], DT, tag="actT", name=f"qTt_{g}")
                load_T(querys_e[ds(g * F, F), :], qTt, F)
                if g == 0:
                    WkT_sb = make_wT(Wk_e, "wk")
                    WvT_sb = make_wT(Wv_e, "wv")
                for dko in range(DK_O):
                    ps = psSC.tile([P, F], F32, tag="ps_sc")
                    for do in range(D_O):
                        nc.tensor.matmul(
                            ps[:],
                            WqT_sb[:, dko, do, :],
                            qTt[:, :, do, :],
                            start=(do == 0),
                            stop=(do == D_O - 1),
                        )
                    nc.scalar.add(
                        qT_sb[:, dko, ds(g * F, F)], ps[:], bq_sb[:, dko : dko + 1]
                    )

            for grp in range(M_GRP):
                ktT = pact.tile([P, 4, D_O, P], DT, tag="actT", name=f"ktT_{grp}")
                load_T(keys_e[ds(grp * F, F), :], ktT, F)
                vtT = pact.tile([P, 4, D_O, P], DT, tag="actT", name=f"vtT_{grp}")
                load_T(values_e[ds(grp * F, F), :], vtT, F)

                # k projection: kT[dk, m-group] += bk
                for dko in range(DK_O):
                    ps = psSC.tile([P, F], F32, tag="ps_sc")
                    for do in range(D_O):
                        nc.tensor.matmul(
                            ps[:],
                            WkT_sb[:, dko, do, :],
                            ktT[:, :, do, :],
                            start=(do == 0),
                            stop=(do == D_O - 1),
                        )
                    nc.scalar.add(
                        kT_sb[:, dko, ds(grp * F, F)], ps[:], bk_sb[:, dko : dko + 1]
                    )

                # v projection: v[m, dv]; vtT tile stationary, reused for both
                # dv chunks (bv applied on host)
                for r in range(4):
                    mo = grp * 4 + r
                    pss = [
                        psSC.tile([P, F], F32, tag="ps_sc", name=f"ps_v_{c}")
                        for c in range(PV_CHUNKS)
                    ]
                    for do in range(D_O):
                        for c in range(PV_CHUNKS):
                            nc.tensor.matmul(
                                pss[c][:],
                                vtT[:, r, do, :],
                                WvT_sb[:, ds(c * 4, 4), do, :],
                                start=(do == 0),
                                stop=(do == D_O - 1),
                            )
                    for c in range(PV_CHUNKS):
                        nc.any.tensor_copy(v_sb[:, mo, ts(c, F)], pss[c][:])

        # ---------------- Phase B: attention blocks ----------------
        with tc.tile_pool(name="mainp", bufs=2) as mp:
            for blk in range(N_BLOCKS):
                # additive mask bias: (mask - 1) * 1e9  ->  {0, -1e9}.
                # mask rows come in as an int32->int8 SWDGE cast-DMA (exact for
                # 0/1) to cut HBM+queue traffic 4x.
                mtile = mp.tile([P, M], I8, tag="mask")
                nc.gpsimd.dma_start(mtile[:], mask_e[ds(blk * P, P), :])
                btile = mp.tile([P, M], F32, tag="maskbias")
                nc.scalar.activation(
                    btile[:],
                    mtile[:],
                    mybir.ActivationFunctionType.Copy,
                    bias=NEG,
                    scale=-NEG,
                )

                stats = mp.tile([P, 2], F32, tag="stats")
                sums = mp.tile([P, 2], F32, tag="sums")
                negmax = mp.tile([P, 1], F32, tag="negmax")
                rsum = mp.tile([P, 1], F32, tag="rsum")
                rinv = mp.tile([P, 1], F32, tag="rinv")
                w16 = mp.tile([P, M], DT, tag="w16")

                # scores: qT block tile stationary, reused across all 4 chunks
                # scores in two [P, 2, F] PSUM pair-tiles (one bank per
                # 512-chunk); stats/exp run on the flattened 1024-wide pairs
                # to halve instruction and semaphore counts in the block tail
                sc_pairs = [
                    psSC.tile([P, 2, F], F32, tag="ps_sc", name=f"ps_sc_{pc}")
                    for pc in range(2)
                ]
                for mc in range(SC_CHUNKS):
                    pc, half = divmod(mc, 2)
                    ps = sc_pairs[pc]
                    for dko in range(DK_O):
                        nc.tensor.matmul(
                            ps[:, half, :],
                            qT_sb[:, dko, ds(blk * P, P)],
                            kT_sb[:, dko, ts(mc, F)],
                            start=(dko == 0),
                            stop=(dko == DK_O - 1),
                        )
                    if half == 1:
                        flat = ps[:].rearrange("p a b -> p (a b)")
                        nc.vector.tensor_add(
                            flat, flat, btile[:, ds(pc * 2 * F, 2 * F)]
                        )
                        nc.vector.reduce_max(
                            stats[:, pc : pc + 1], flat, axis=mybir.AxisListType.X
                        )
                nc.vector.reduce_max(
                    negmax[:], stats[:], axis=mybir.AxisListType.X, negate=True
                )

                for pc in range(2):
                    nc.scalar.activation(
                        w16[:, ds(pc * 2 * F, 2 * F)],
                        sc_pairs[pc][:].rearrange("p a b -> p (a b)"),
                        mybir.ActivationFunctionType.Exp,
                        bias=negmax[:, 0:1],
                        scale=1.0,
                        accum_out=sums[:, pc : pc + 1],
                    )
                nc.vector.reduce_sum(rsum[:], sums[:], axis=mybir.AxisListType.X)
                nc.vector.reciprocal(rinv[:], rsum[:])

                # X-bar transpose of the probability tiles: [n, m] -> [m_i, m_o, n]
                # (in phase B the X-bar stays off the PE's critical path; the
                # only other DMAs here are SWDGE mask/out, so mode-transition
                # serialization has slack)
                wT = mp.tile([P, M_BLOCKS, P], DT, tag="wT")
                for h in range(2):
                    nc.sync.dma_start(
                        wT[:, ds(h * 8, 8), :],
                        w16[:, ds(h * 1024, 1024)],
                        transpose=True,
                    )

                # PV: out[n-block, dv] = wT.T @ v; wT tile stationary per mo
                pv = psPV.tile([P, PV_CHUNKS, F], F32, tag="ps_pv")
                for mo in range(M_BLOCKS):
                    for c in range(PV_CHUNKS):
                        nc.tensor.matmul(
                            pv[:, c, :],
                            wT[:, mo, :],
                            v_sb[:, mo, ts(c, F)],
                            start=(mo == 0),
                            stop=(mo == M_BLOCKS - 1),
                        )
                outt = mp.tile([P, DV], F32, tag="outt")
                for c in range(PV_CHUNKS):
                    nc.vector.tensor_scalar_mul(
                        outt[:, ts(c, F)], pv[:, c, :], rinv[:, 0:1]
                    )
                nc.gpsimd.dma_start(out_e[ds(blk * P, P), :], outt[:])

    nc.compile()
    return nc


_CACHE = {}


def _get_nc():
    if "nc" not in _CACHE:
        _CACHE["nc"] = build()
    return _CACHE["nc"]


def run(inputs, trace=False, trace_kwargs=None):
    nc = _get_nc()
    querys = np.ascontiguousarray(np.asarray(inputs["querys"], dtype=np.float32))
    keys = np.ascontiguousarray(np.asarray(inputs["keys"], dtype=np.float32))
    values = np.ascontiguousarray(np.asarray(inputs["values"], dtype=np.float32))
    mask = np.ascontiguousarray(np.asarray(inputs["mask"], dtype=np.int32))
    shared = {
        "mask": mask,
        "Wq": np.asarray(inputs["Wq"], dtype=np.float32),
        "Wk": np.asarray(inputs["Wk"], dtype=np.float32),
        "Wv": np.asarray(inputs["Wv"], dtype=np.float32),
        "bq": np.asarray(inputs["bq"], dtype=np.float32),
        "bk": np.asarray(inputs["bk"], dtype=np.float32),
    }
    in_maps = [
        {
            "querys": querys[b],
            "keys": keys[b],
            "values": values[b],
            **shared,
        }
        for b in range(B)
    ]
    res = run_bass_kernel_spmd(
        nc,
        in_maps,
        list(range(B)),
        trace=trace,
        **(trace_kwargs or {}),
    )
    out = np.stack([res.results[b]["out"] for b in range(B)]).astype(np.float32)
    # bv folded in on the host: softmax rows sum to 1, so W @ (v + bv) = W @ v + bv
    out += np.asarray(inputs["bv"], dtype=np.float32)[None, None, :]
    return out, res


def kernel(**inputs) -> np.ndarray:
    out, _ = run(inputs, trace=False)
    return out


if __name__ == "__main__":
    nc = _get_nc()
    print("built + compiled OK")


# revision 28
# speedup vs baseline: 1.1686x; 1.1686x over previous
"""Trainium2 Bass kernel for nn_Attention_5480378270188.

Single-layer attention: q/k/v linear projections (torch Linear convention),
scores = q @ k^T (no 1/sqrt(d) scale), additive -1e9 mask, softmax over keys,
out = weights @ v.

Shapes (hardcoded): B=8, N=M=2048, D_MODEL=D_K=D_V=1024, fp32 inputs.

Sharding: data-parallel over batch — core b computes batch element b.
mask / W / biases are replicated to all 8 cores. No collectives.

On-device dtype strategy: all TensorE operands fp16 (full PE rate), fp32 PSUM
accumulation, softmax entirely in fp32. Operand transposes run on the DMA
X-bar (16-bit) out of an fp16 DRAM bounce produced by HWDGE staging + an
on-chip DVE/ACT cast — the PE does nothing but matmuls. bq/bk are applied
on-device (per-partition ACT bias fused into the projection PSUM->SBUF
copies). bv is applied on the host: softmax rows sum to 1, so
softmax(s) @ (v + bv) == softmax(s) @ v + bv exactly.
"""

import sys

for _p in ("/opt/trn_rl_repo", "/opt/pypackages"):
    if _p not in sys.path:
        sys.path.insert(0, _p)

from contextlib import ExitStack

import numpy as np

import concourse.bass as bass
import concourse.tile as tile
from concourse import bacc, mybir
from concourse.bass import ds, ts
from concourse.bass_utils import run_bass_kernel_spmd
from concourse.masks import make_identity

P = 128
B = 8
N = 2048  # queries
M = 2048  # keys
D = 1024  # d_model
DK = 1024  # key/query dim
DV = 1024  # value dim
F = 512  # matmul moving free dim
DT = mybir.dt.float16
F32 = mybir.dt.float32
I32 = mybir.dt.int32
I8 = mybir.dt.int8

NEG = -1.0e9

N_BLOCKS = N // P  # 16
M_BLOCKS = M // P  # 16
D_O = D // P  # 8
DK_O = DK // P  # 8
N_MEGA = N // F  # 4 query mega-blocks (512 rows)
M_GRP = M // F  # 4 key groups (512 rows)
SC_CHUNKS = M // F  # 4 score chunks per row-block
PV_CHUNKS = DV // F  # 2


def build():
    nc = bacc.Bacc("TRN2", target_bir_lowering=False, debug=False)

    querys_e = nc.dram_tensor("querys", [N, D], F32, kind="ExternalInput").ap()
    keys_e = nc.dram_tensor("keys", [M, D], F32, kind="ExternalInput").ap()
    values_e = nc.dram_tensor("values", [M, D], F32, kind="ExternalInput").ap()
    mask_e = nc.dram_tensor("mask", [N, M], I32, kind="ExternalInput").ap()
    Wq_e = nc.dram_tensor("Wq", [DK, D], F32, kind="ExternalInput").ap()
    Wk_e = nc.dram_tensor("Wk", [DK, D], F32, kind="ExternalInput").ap()
    Wv_e = nc.dram_tensor("Wv", [DV, D], F32, kind="ExternalInput").ap()
    bq_e = nc.dram_tensor("bq", [DK], F32, kind="ExternalInput").ap()
    bk_e = nc.dram_tensor("bk", [DK], F32, kind="ExternalInput").ap()
    out_e = nc.dram_tensor("out", [N, DV], F32, kind="ExternalOutput").ap()

    with tile.TileContext(nc) as tc, ExitStack() as ctx:
        const = ctx.enter_context(tc.tile_pool(name="const", bufs=1))
        persist = ctx.enter_context(tc.tile_pool(name="persist", bufs=1))
        dram = ctx.enter_context(tc.tile_pool(name="dram", bufs=1, space="DRAM"))
        psSC = ctx.enter_context(tc.tile_pool(name="psSC", bufs=6, space="PSUM"))
        psPV = ctx.enter_context(tc.tile_pool(name="psPV", bufs=1, space="PSUM"))

        ident16 = const.tile([P, P], DT, tag="id16")
        make_identity(nc, ident16[:])

        bq_sb = const.tile([P, DK_O], F32, tag="bq")
        nc.sync.dma_start(bq_sb[:], bq_e.rearrange("(o p) -> p o", p=P))
        bk_sb = const.tile([P, DK_O], F32, tag="bk")
        nc.sync.dma_start(bk_sb[:], bk_e.rearrange("(o p) -> p o", p=P))

        # persistent fp16 operands for the attention matmuls
        kT_sb = persist.tile([P, DK_O, M], DT, tag="kT")  # [dk_i, dk_o, m]
        v_sb = persist.tile([P, M_BLOCKS, DV], DT, tag="v")  # [m_i, m_o, dv]
        qT_sb = persist.tile([P, DK_O, N], DT, tag="qT")  # [dk_i, dk_o, n]

        # ---------------- Phase A: all projections ----------------
        with (
            tc.tile_pool(name="phW", bufs=1) as pw,
            tc.tile_pool(name="phA", bufs=5) as pa,
            tc.tile_pool(name="phT", bufs=2) as pact,
        ):
            # Transposed fp16 operands: HWDGE stage-in (fp32) -> on-chip
            # cast (DVE/ACT) -> SBUF->SBUF X-bar transpose per 128-row tile,
            # alternating between the two HWDGE queues. No DRAM bounce.
            def load_T(src_rows, dst, rows):
                """src_rows: [rows, D] fp32 DRAM -> dst [P, rows//P, D_O, P] fp16.

                Per 512 rows: one SWDGE cast-DMA (fp32->fp16 in flight) into a
                staging tile, then PE transposes (fp16, batched 4 per PSUM
                tile) with ACT/DVE copy-back. No X-bar: every DMA in the
                machine serializes globally against X-bar-transpose DMAs
                (~10us dead time per mode transition), which starves the PE;
                PE transposes cost ~130ns each and overlap the DMA stream.
                """
                for ch in range(rows // F):
                    src = src_rows[ds(ch * F, F), :]
                    st16 = pa.tile([P, 4, D], DT, tag="st16")
                    nc.gpsimd.dma_start(
                        st16[:], src.rearrange("(ro p) d -> p ro d", p=P)
                    )
                    for rt in range(4):
                        for h in range(2):
                            ps = psSC.tile([P, 4, P], DT, tag="ps_sc", name="ps_t")
                            for j in range(4):
                                db = h * 4 + j
                                nc.tensor.transpose(
                                    ps[:, j, :],
                                    st16[:, rt, ts(db, P)],
                                    ident16[:],
                                )
                            nc.any.tensor_copy(
                                dst[:, ch * 4 + rt, ds(h * 4, 4), :], ps[:]
                            )

            def make_wT(w_ext, tag):
                # [dk_i_rowtile(=dko), d_o, dk_i] — weight tile for (do, dko)
                # is wT[:, dko, do, :]
                wT = pw.tile([P, DK_O, D_O, P], DT, tag=f"{tag}T")
                load_T(w_ext[:], wT, DK)
                return wT

            WqT_sb = make_wT(Wq_e, "wq")

            WkT_sb = None
            WvT_sb = None

            # q projections (fill the PE while k/v staging is in flight)
            for g in range(N_MEGA):
                qTt = pact.tile([P, 4, D_O, P], DT, tag="actT", name=f"qTt_{g}")
                load_T(querys_e[ds(g * F, F), :], qTt, F)
                if g == 0:
                    WkT_sb = make_wT(Wk_e, "wk")
                    WvT_sb = make_wT(Wv_e, "wv")
                for dko in range(DK_O):
                    ps = psSC.tile([P, F], F32, tag="ps_sc")
                    for do in range(D_O):
                        nc.tensor.matmul(
                            ps[:],
                            WqT_sb[:, dko, do, :],
                            qTt[:, :, do, :],
                            start=(do == 0),
                            stop=(do == D_O - 1),
                        )
                    nc.scalar.add(
                        qT_sb[:, dko, ds(g * F, F)], ps[:], bq_sb[:, dko : dko + 1]
                    )

            for grp in range(M_GRP):
                ktT = pact.tile([P, 4, D_O, P], DT, tag="actT", name=f"ktT_{grp}")
                load_T(keys_e[ds(grp * F, F), :], ktT, F)
                vtT = pact.tile([P, 4, D_O, P], DT, tag="actT", name=f"vtT_{grp}")
                load_T(values_e[ds(grp * F, F), :], vtT, F)

                # k projection: kT[dk, m-group] += bk
                for dko in range(DK_O):
                    ps = psSC.tile([P, F], F32, tag="ps_sc")
                    for do in range(D_O):
                        nc.tensor.matmul(
                            ps[:],
                            WkT_sb[:, dko, do, :],
                            ktT[:, :, do, :],
                            start=(do == 0),
                            stop=(do == D_O - 1),
                        )
                    nc.scalar.add(
                        kT_sb[:, dko, ds(grp * F, F)], ps[:], bk_sb[:, dko : dko + 1]
                    )

                # v projection: v[m, dv]; vtT tile stationary, reused for both
                # dv chunks (bv applied on host)
                for r in range(4):
                    mo = grp * 4 + r
                    pss = [
                        psSC.tile([P, F], F32, tag="ps_sc", name=f"ps_v_{c}")
                        for c in range(PV_CHUNKS)
                    ]
                    for do in range(D_O):
                        for c in range(PV_CHUNKS):
                            nc.tensor.matmul(
                                pss[c][:],
                                vtT[:, r, do, :],
                                WvT_sb[:, ds(c * 4, 4), do, :],
                                start=(do == 0),
                                stop=(do == D_O - 1),
                            )
                    for c in range(PV_CHUNKS):
                        nc.any.tensor_copy(v_sb[:, mo, ts(c, F)], pss[c][:])

        # ---------------- Phase B: attention blocks ----------------
        with tc.tile_pool(name="mainp", bufs=2) as mp:
            for blk in range(N_BLOCKS):
                # additive mask bias: (mask - 1) * 1e9  ->  {0, -1e9}.
                # mask rows come in as an int32->int8 SWDGE cast-DMA (exact for
                # 0/1) to cut HBM+queue traffic 4x.
                mtile = mp.tile([P, M], I8, tag="mask")
                nc.gpsimd.dma_start(mtile[:], mask_e[ds(blk * P, P), :])
                btile = mp.tile([P, M], F32, tag="maskbias")
                nc.scalar.activation(
                    btile[:],
                    mtile[:],
                    mybir.ActivationFunctionType.Copy,
                    bias=NEG,
                    scale=-NEG,
                )

                stats = mp.tile([P, SC_CHUNKS], F32, tag="stats")
                sums = mp.tile([P, SC_CHUNKS], F32, tag="sums")
                negmax = mp.tile([P, 1], F32, tag="negmax")
                rsum = mp.tile([P, 1], F32, tag="rsum")
                rinv = mp.tile([P, 1], F32, tag="rinv")
                w16 = mp.tile([P, M], DT, tag="w16")

                # scores: qT block tile stationary, reused across all 4 chunks
                sc_tiles = [
                    psSC.tile([P, F], F32, tag="ps_sc", name=f"ps_sc_{mc}")
                    for mc in range(SC_CHUNKS)
                ]
                for mc in range(SC_CHUNKS):
                    for dko in range(DK_O):
                        nc.tensor.matmul(
                            sc_tiles[mc][:],
                            qT_sb[:, dko, ds(blk * P, P)],
                            kT_sb[:, dko, ts(mc, F)],
                            start=(dko == 0),
                            stop=(dko == DK_O - 1),
                        )
                    nc.vector.tensor_add(
                        sc_tiles[mc][:], sc_tiles[mc][:], btile[:, ts(mc, F)]
                    )
                    nc.vector.reduce_max(
                        stats[:, mc : mc + 1], sc_tiles[mc][:], axis=mybir.AxisListType.X
                    )
                nc.vector.reduce_max(
                    negmax[:], stats[:], axis=mybir.AxisListType.X, negate=True
                )

                for mc in range(SC_CHUNKS):
                    nc.scalar.activation(
                        w16[:, ts(mc, F)],
                        sc_tiles[mc][:],
                        mybir.ActivationFunctionType.Exp,
                        bias=negmax[:, 0:1],
                        scale=1.0,
                        accum_out=sums[:, mc : mc + 1],
                    )
                nc.vector.reduce_sum(rsum[:], sums[:], axis=mybir.AxisListType.X)
                nc.vector.reciprocal(rinv[:], rsum[:])

                # X-bar transpose of the probability tiles: [n, m] -> [m_i, m_o, n]
                # (in phase B the X-bar stays off the PE's critical path; the
                # only other DMAs here are SWDGE mask/out, so mode-transition
                # serialization has slack)
                wT = mp.tile([P, M_BLOCKS, P], DT, tag="wT")
                for h in range(2):
                    nc.sync.dma_start(
                        wT[:, ds(h * 8, 8), :],
                        w16[:, ds(h * 1024, 1024)],
                        transpose=True,
                    )

                # PV: out[n-block, dv] = wT.T @ v; wT tile stationary per mo
                pv = psPV.tile([P, PV_CHUNKS, F], F32, tag="ps_pv")
                for mo in range(M_BLOCKS):
                    for c in range(PV_CHUNKS):
                        nc.tensor.matmul(
                            pv[:, c, :],
                            wT[:, mo, :],
                            v_sb[:, mo, ts(c, F)],
                            start=(mo == 0),
                            stop=(mo == M_BLOCKS - 1),
                        )
                outt = mp.tile([P, DV], F32, tag="outt")
                for c in range(PV_CHUNKS):
                    nc.vector.tensor_scalar_mul(
                        outt[:, ts(c, F)], pv[:, c, :], rinv[:, 0:1]
                    )
                nc.gpsimd.dma_start(out_e[ds(blk * P, P), :], outt[:])

    nc.compile()
    return nc


_CACHE = {}


def _get_nc():
    if "nc" not in _CACHE:
        _CACHE["nc"] = build()
    return _CACHE["nc"]


def run(inputs, trace=False, trace_kwargs=None):
    nc = _get_nc()
    querys = np.ascontiguousarray(np.asarray(inputs["querys"], dtype=np.float32))
    keys = np.ascontiguousarray(np.asarray(inputs["keys"], dtype=np.float32))
    values = np.ascontiguousarray(np.asarray(inputs["values"], dtype=np.float32))
    mask = np.ascontiguousarray(np.asarray(inputs["mask"], dtype=np.int32))
    shared = {
        "mask": mask,
        "Wq": np.asarray(inputs["Wq"], dtype=np.float32),
        "Wk": np.asarray(inputs["Wk"], dtype=np.float32),
        "Wv": np.asarray(inputs["Wv"], dtype=np.float32),
        "bq": np.asarray(inputs["bq"], dtype=np.float32),
        "bk": np.asarray(inputs["bk"], dtype=np.float32),
    }
    in_maps = [
        {
            "querys": querys[b],
            "keys": keys[b],
            "values": values[b],
            **shared,
        }
        for b in range(B)
    ]
    res = run_bass_kernel_spmd(
        nc,
        in_maps,
        list(range(B)),
        trace=trace,
        **(trace_kwargs or {}),
    )
    out = np.stack([res.results[b]["out"] for b in range(B)]).astype(np.float32)
    # bv folded in on the host: softmax rows sum to 1, so W @ (v + bv) = W @ v + bv
    out += np.asarray(inputs["bv"], dtype=np.float32)[None, None, :]
    return out, res


def kernel(**inputs) -> np.ndarray:
    out, _ = run(inputs, trace=False)
    return out


if __name__ == "__main__":
    nc = _get_nc()
    print("built + compiled OK")
